# revision 1
# baseline (speedup 1.0000x reference)
"""Fused rotary QK-projection + normalized dot-product attention softmax.

Computes softmax((q_hat @ k_hat^T) / 64) for q,k = L2-normalized rotary
projections of x, sharded over 8 NeuronCores as (batch x head-pair):
core c -> batch c//4, heads (2*(c%4), 2*(c%4)+1). No cross-core comms.

Self-contained: hardcodes shapes b=2, n=2048, dim=512, h=8, d=64.
"""

import numpy as np
import ml_dtypes

B = 2
N = 2048
C = 512           # model dim (contraction for projection)
H = 8             # heads
D = 64            # head dim
HPC = 2           # heads per core
NCORES = 8
KC = C // 128     # 4 contraction chunks of 128
NJ = N // 512     # 4 chain chunks of 512
NT = N // 128     # 16 q row-tiles

_CACHE = {}


def _setup_act_tables():
    """Point walrus at an act_info.json tweaked so Ln and Exp both resolve
    to natural_log_exp_and_others (one shared ACT table set -> no ~1.3us
    table reloads between Ln and Exp activations). Set order/indices are
    kept identical; only the per-func set choice is steered."""
    import os
    import json
    import tempfile
    from pathlib import Path

    if os.environ.get("BASS_ACT_ROOT_JSON_PATH"):
        return
    from neuronxcc.driver.Job import Job

    src_dir = Path(Job.getPackageDir()) / "pwp" / "pwp_bin_trainium"
    src_json = src_dir / "act_info.json"
    if not src_json.exists():
        return
    info = json.loads(src_json.read_text())
    sets = info.get("act_func_sets", [])
    names = [s.get("name") for s in sets]
    if "natural_log_exp_and_others" not in names:
        return
    for s in sets:
        if s.get("name") == "exp_and_others":
            s.get("act", {}).pop("exp", None)
        elif s.get("name") == "natural_log":
            s.get("act", {}).pop("ln", None)
    dst_dir = Path(tempfile.mkdtemp(prefix="pwp_act_"))
    for f in src_dir.iterdir():
        if f.name != "act_info.json":
            (dst_dir / f.name).symlink_to(f)
    (dst_dir / "act_info.json").write_text(json.dumps(info))
    os.environ["BASS_ACT_ROOT_JSON_PATH"] = str(dst_dir / "act_info.json")


def _build_nc():
    import concourse.mybir as mybir
    import concourse.tile as tile
    from concourse import bacc

    _setup_act_tables()

    dt = mybir.dt
    f32, bf16 = dt.float32, dt.bfloat16
    AF = mybir.ActivationFunctionType

    nc = bacc.Bacc(None)
    # partition-major host layouts -> contiguous per-partition DMA segments
    xT = nc.dram_tensor("xT", [128, NJ, KC, 512], bf16, kind="ExternalInput")
    wq = nc.dram_tensor("wq", [128, HPC, KC, 128], bf16, kind="ExternalInput")
    wr = nc.dram_tensor("wr", [128, HPC, KC, 128], bf16, kind="ExternalInput")
    cosr = nc.dram_tensor("cosr", [128, N], bf16, kind="ExternalInput")
    sinr = nc.dram_tensor("sinr", [128, N], bf16, kind="ExternalInput")
    maskt = nc.dram_tensor("maskt", [128, 2], bf16, kind="ExternalInput")
    out = nc.dram_tensor("out", [HPC, N, N], f32, kind="ExternalOutput")
    # bounce rows: per-head 1/|q_n| (partition-gathered) and 1/|k_n|
    # (partition-broadcast)
    rqd = nc.dram_tensor("rqd", [HPC, N], bf16)
    rkd = nc.dram_tensor("rkd", [HPC, N], bf16)

    with tile.TileContext(nc) as tc:
        with (
            tc.tile_pool(name="singles", bufs=1) as singles,
            tc.tile_pool(name="chain", bufs=2) as chain_pool,
            tc.tile_pool(name="persist", bufs=2) as persist,
            tc.tile_pool(name="exp", bufs=4) as exp_pool,
            tc.tile_pool(name="outp", bufs=6) as out_pool,
            tc.tile_pool(name="small", bufs=8) as small,
            tc.tile_pool(name="pchain", bufs=2, space="PSUM") as pchain,
            tc.tile_pool(name="psc", bufs=2, space="PSUM") as psc,
        ):
            wqt = singles.tile([128, HPC, KC, 128], bf16)
            nc.sync.dma_start(out=wqt[:], in_=wq[:])
            wrt = singles.tile([128, HPC, KC, 128], bf16)
            nc.sync.dma_start(out=wrt[:], in_=wr[:])
            mask = singles.tile([128, 2], bf16)
            nc.sync.dma_start(out=mask[:], in_=maskt[:])
            cost = singles.tile([128, N], bf16)
            sint = singles.tile([128, N], bf16)
            xt = singles.tile([128, NJ, KC, 512], bf16)
            for j in range(NJ):
                nc.sync.dma_start(out=xt[:, j, :, :], in_=xT[:, j, :, :])
            for j in range(NJ):
                js = slice(j * 512, (j + 1) * 512)
                nc.sync.dma_start(out=cost[:, js], in_=cosr[:, js])
                nc.sync.dma_start(out=sint[:, js], in_=sinr[:, js])

            # per-head persistent tiles (indexed by head)
            qr_t, kt_t, nsq_t, rqs_t = {}, {}, {}, {}

            def chain_start(t):
                # [e, n] layout: partitions = 64 q-dims then 64 k-dims.
                qr_t[t] = persist.tile([128, N], bf16, tag="qr", name=f"qr{t}")
                kt_t[t] = persist.tile([64, N], bf16, tag="kt", name=f"kt{t}")
                nsq_t[t] = persist.tile([2, N], f32, tag="nsq", name=f"nsq{t}")

            def chain_chunk(t, j):
                qr, nsq_sb = qr_t[t], nsq_t[t]
                js = slice(j * 512, (j + 1) * 512)
                qk_ps = pchain.tile([128, 512], f32, tag="pq")
                rot_ps = pchain.tile([128, 512], f32, tag="pr")
                for k in range(KC):
                    nc.tensor.matmul(
                        qk_ps[:], lhsT=wqt[:, t, k, :], rhs=xt[:, j, k, :],
                        start=(k == 0), stop=(k == KC - 1),
                    )
                for k in range(KC):
                    nc.tensor.matmul(
                        rot_ps[:], lhsT=wrt[:, t, k, :], rhs=xt[:, j, k, :],
                        start=(k == 0), stop=(k == KC - 1),
                    )
                # rotary: qr = qk*cos + rot(qk)*sin
                t1 = chain_pool.tile([128, 512], bf16, tag="t1")
                nc.vector.tensor_mul(t1[:], qk_ps[:], cost[:, js])
                t2 = chain_pool.tile([128, 512], bf16, tag="t2")
                nc.vector.tensor_mul(t2[:], rot_ps[:], sint[:, js])
                nc.vector.tensor_add(qr[:, js], t1[:], t2[:])
                # squared L2 norms along d (partition dim) via mask matmul
                sq = chain_pool.tile([128, 512], bf16, tag="sq")
                nc.vector.tensor_mul(sq[:], qr[:, js], qr[:, js])
                nsq_ps = pchain.tile([2, 512], f32, tag="pq")
                nc.tensor.matmul(
                    nsq_ps[:], lhsT=mask[:], rhs=sq[:], start=True, stop=True
                )
                nc.vector.tensor_copy(nsq_sb[:, js], nsq_ps[:])

            def chain_norm(t):
                # rinorm = nsq^-0.5 = exp(-0.5*ln(nsq)), batched per head
                # (2 ACT table switches); the Exp side is chunked so the
                # downstream k-normalize pipeline starts per chunk.
                nsq_sb = nsq_t[t]
                rin = chain_pool.tile([2, N], bf16, tag="rin", name=f"rin{t}")
                lnn = chain_pool.tile([2, N], f32, tag="lnn", name=f"lnn{t}")
                nc.scalar.activation(out=lnn[:], in_=nsq_sb[:], func=AF.Ln)
                nc.scalar.activation(out=rin[:], in_=lnn[:], func=AF.Exp, scale=-0.5)
                nc.sync.dma_start(out=rqd[t, :], in_=rin[0:1, :])
                nc.sync.dma_start(out=rkd[t, :], in_=rin[1:2, :])

            def chain_knorm(t, j):
                # kt = (k-half of qr, shifted to base partition 0) * 1/|k|
                qr, kt = qr_t[t], kt_t[t]
                js = slice(j * 512, (j + 1) * 512)
                ks = chain_pool.tile([64, 512], bf16, tag="ks")
                nc.sync.dma_start(out=ks[:], in_=qr[64:128, js])
                bc = chain_pool.tile([64, 512], bf16, tag="bc")
                nc.sync.dma_start(out=bc[:], in_=rkd[t:t + 1, js].to_broadcast([64, 512]))
                nc.vector.tensor_mul(kt[:, js], ks[:], bc[:])

            def chain_finish(t):
                # per-q-tile exp scales: gather 1/|q| across partitions, /64
                rqt = small.tile([128, NT], bf16, tag="rqt")
                nc.sync.dma_start(
                    out=rqt[:], in_=rqd[t].rearrange("(i p) -> p i", p=128)
                )
                rqs = small.tile([128, NT], f32, tag="rqs", name=f"rqs{t}")
                nc.vector.tensor_scalar_mul(rqs[:], rqt[:], 1.0 / D)
                rqs_t[t] = rqs

            def scores_tile(t, i):
                qr, kt, rqs = qr_t[t], kt_t[t], rqs_t[t]
                isl = slice(i * 128, (i + 1) * 128)
                et = exp_pool.tile([128, N], f32, tag="et")
                sums = small.tile([128, 2], f32, tag="sums")
                for hlf in range(2):
                    hs = slice(hlf * 1024, (hlf + 1) * 1024)
                    sc_ps = psc.tile([128, 1024], f32, tag="sc")
                    for j2 in range(2):
                        js2 = slice(hlf * 1024 + j2 * 512, hlf * 1024 + (j2 + 1) * 512)
                        nc.tensor.matmul(
                            sc_ps[:, j2 * 512:(j2 + 1) * 512],
                            lhsT=qr[0:64, isl], rhs=kt[:, js2],
                            start=True, stop=True,
                        )
                    nc.scalar.activation(
                        out=et[:, hs], in_=sc_ps[:], func=AF.Exp,
                        scale=rqs[:, i:i + 1], accum_out=sums[:, hlf:hlf + 1],
                    )
                ssum = small.tile([128, 1], f32, tag="ssum")
                nc.vector.tensor_tensor(
                    out=ssum[:], in0=sums[:, 0:1], in1=sums[:, 1:2],
                    op=mybir.AluOpType.add,
                )
                rs = small.tile([128, 1], f32, tag="rs")
                nc.vector.reciprocal(out=rs[:], in_=ssum[:])
                ot = out_pool.tile([128, N], f32, tag="ot")
                nc.vector.tensor_scalar_mul(ot[:], et[:], rs[:])
                nc.sync.dma_start(out=out[t, isl, :], in_=ot[:])

            # software pipeline: head-1 chain interleaved into head-0 scores
            chain_start(0)
            for j in range(NJ):
                chain_chunk(0, j)
            chain_norm(0)
            chain_finish(0)
            for j in range(NJ):
                chain_knorm(0, j)
            chain_start(1)
            for i in range(NT):
                scores_tile(0, i)
                if i in (1, 3, 5, 7):
                    chain_chunk(1, (i - 1) // 2)
                elif i == 9:
                    chain_norm(1)
                    chain_finish(1)
                    for j in range(NJ):
                        chain_knorm(1, j)
            for i in range(NT):
                scores_tile(1, i)

    nc.compile()
    return nc


def _get_nc():
    if "nc" not in _CACHE:
        _CACHE["nc"] = _build_nc()
    return _CACHE["nc"]


def _prep_inputs(x, rotary_cos, rotary_sin, W_qk):
    bf16 = ml_dtypes.bfloat16
    x = np.asarray(x, dtype=np.float32)
    cos = np.asarray(rotary_cos, dtype=np.float32)
    sin = np.asarray(rotary_sin, dtype=np.float32)
    W = np.asarray(W_qk, dtype=np.float32)

    cosr = np.concatenate([cos.T, cos.T], axis=0).astype(bf16)  # [128, N]
    sinr = np.concatenate([sin.T, sin.T], axis=0).astype(bf16)
    maskt = np.zeros((128, 2), dtype=bf16)
    maskt[0:64, 0] = 1.0
    maskt[64:128, 1] = 1.0

    # per-head weight lhsT chunks (and rotate_half-permuted variant),
    # stored partition-major: [p, head, kc, m]
    wq_h = np.empty((H, KC, 128, 128), dtype=np.float32)
    wr_h = np.empty((H, KC, 128, 128), dtype=np.float32)
    for h in range(H):
        wcat = np.concatenate(
            [W[h * D:(h + 1) * D], W[C + h * D:C + (h + 1) * D]], axis=0
        )  # [128, 512]
        wrot = np.empty_like(wcat)
        wrot[0:32] = -wcat[32:64]
        wrot[32:64] = wcat[0:32]
        wrot[64:96] = -wcat[96:128]
        wrot[96:128] = wcat[64:96]
        wq_h[h] = wcat.T.reshape(KC, 128, 128)
        wr_h[h] = wrot.T.reshape(KC, 128, 128)

    # xT partition-major chunked: [p, j, kc, nn]
    xTb = []
    for b in range(B):
        xT = x[b].T  # [C, N]
        xTb.append(np.ascontiguousarray(
            xT.reshape(KC, 128, NJ, 512).transpose(1, 2, 0, 3)
        ).astype(bf16))

    in_maps = []
    for core in range(NCORES):
        b = core // 4
        h0 = (core % 4) * HPC
        wqc = np.ascontiguousarray(
            wq_h[h0:h0 + HPC].transpose(2, 0, 1, 3)
        ).astype(bf16)  # [128, HPC, KC, 128]
        wrc = np.ascontiguousarray(
            wr_h[h0:h0 + HPC].transpose(2, 0, 1, 3)
        ).astype(bf16)
        in_maps.append({
            "xT": xTb[b],
            "wq": wqc,
            "wr": wrc,
            "cosr": cosr,
            "sinr": sinr,
            "maskt": maskt,
        })
    return in_maps


def run(x, rotary_cos, rotary_sin, W_qk, trace=False):
    from concourse.bass_utils import run_bass_kernel_spmd

    nc = _get_nc()
    in_maps = _prep_inputs(x, rotary_cos, rotary_sin, W_qk)
    res = run_bass_kernel_spmd(nc, in_maps, list(range(NCORES)), trace=trace)
    full = np.empty((B, H, N, N), dtype=np.float32)
    for core in range(NCORES):
        b = core // 4
        h0 = (core % 4) * HPC
        for t in range(HPC):
            full[b, h0 + t] = res.results[core]["out"][t]
    return full, res


def kernel(x, rotary_cos, rotary_sin, W_qk):
    full, _ = run(x, rotary_cos, rotary_sin, W_qk, trace=False)
    return full



# revision 4
# speedup vs baseline: 1.0011x; 1.0011x over previous
"""Fused rotary QK-projection + normalized dot-product attention softmax.

Computes softmax((q_hat @ k_hat^T) / 64) for q,k = L2-normalized rotary
projections of x, sharded over 8 NeuronCores as (batch x head-pair):
core c -> batch c//4, heads (2*(c%4), 2*(c%4)+1). No cross-core comms.

Since |q_hat . k_hat| <= 1, scores x = (q_hat.k_hat)/64 lie in [-1/64, 1/64]
and exp(x) = 1 + x to ~1.2e-4 relative, so
    softmax_i(x) ~= (1 + x_i) / (N + sum_j x_j)
(error ~2e-4 << the 2e-2 gate). This removes the Exp/softmax pass entirely:
each output tile is a single fused scale+bias of the scores matmul,
    out_ij = s_i * (q_i . k^_j) + r_i,
      r_i = 1/(N + (rq_i/64) * (q_i . Ksum)),  s_i = r_i * rq_i / 64,
with rq = 1/|q|, k^ = k/|k|, Ksum = sum_j k^_j. The row sum comes from one
tiny matmul against Ksum instead of a full reduction.

Output is encoded as fp8e4 deltas: dev stores (out - 1/N) * 2^24 (|.| <= 129
by Cauchy-Schwarz); host decodes fp8 * 2^-24 + 1/N. This quarters the
dominant DMA-out traffic vs fp32.

Self-contained: hardcodes shapes b=2, n=2048, dim=512, h=8, d=64.
"""

import numpy as np
import ml_dtypes

B = 2
N = 2048
C = 512           # model dim (contraction for projection)
H = 8             # heads
D = 64            # head dim
HPC = 2           # heads per core
NCORES = 8
KC = C // 128     # 4 contraction chunks of 128
NJ = N // 512     # 4 chain chunks of 512
NT = N // 128     # 16 q row-tiles

OSCALE = 2.0 ** 24          # fp8 delta scale
MASK_Q = 2.0 ** -36         # rsqrt(nsq*MASK_Q) = (1/|q|) * 2^24 / 64

_CACHE = {}


def _build_nc():
    import concourse.mybir as mybir
    import concourse.tile as tile
    from concourse import bacc

    dt = mybir.dt
    f32, bf16, f8 = dt.float32, dt.bfloat16, dt.float8e4
    AF = mybir.ActivationFunctionType
    OP = mybir.AluOpType

    nc = bacc.Bacc(None)
    # partition-major host layouts -> contiguous per-partition DMA segments
    xT = nc.dram_tensor("xT", [128, NJ, KC, 512], bf16, kind="ExternalInput")
    wq = nc.dram_tensor("wq", [128, HPC, KC, 128], bf16, kind="ExternalInput")
    wr = nc.dram_tensor("wr", [128, HPC, KC, 128], bf16, kind="ExternalInput")
    cosr = nc.dram_tensor("cosr", [128, N], bf16, kind="ExternalInput")
    sinr = nc.dram_tensor("sinr", [128, N], bf16, kind="ExternalInput")
    maskt = nc.dram_tensor("maskt", [128, 2], bf16, kind="ExternalInput")
    out = nc.dram_tensor("out", [HPC, N, N], f8, kind="ExternalOutput")
    # bounce rows: per-head squared norms (row 0: |q|^2 * MASK_Q, row 1:
    # |k|^2), gathered back partition-major; per-head 1/|k| row for the
    # k-normalize broadcast
    nsqd = nc.dram_tensor("nsqd", [HPC, 2, N], f32)
    rkd = nc.dram_tensor("rkd", [HPC, N], bf16)

    with tile.TileContext(nc) as tc:
        with (
            tc.tile_pool(name="singles", bufs=1) as singles,
            tc.tile_pool(name="chain", bufs=2) as chain_pool,
            tc.tile_pool(name="persist", bufs=2) as persist,
            tc.tile_pool(name="outp", bufs=6) as out_pool,
            tc.tile_pool(name="small", bufs=8) as small,
            tc.tile_pool(name="pchain", bufs=2, space="PSUM") as pchain,
            tc.tile_pool(name="psc", bufs=2, space="PSUM") as psc,
        ):
            wqt = singles.tile([128, HPC, KC, 128], bf16)
            nc.sync.dma_start(out=wqt[:], in_=wq[:])
            wrt = singles.tile([128, HPC, KC, 128], bf16)
            nc.sync.dma_start(out=wrt[:], in_=wr[:])
            mask = singles.tile([128, 2], bf16)
            nc.sync.dma_start(out=mask[:], in_=maskt[:])
            cost = singles.tile([128, N], bf16)
            sint = singles.tile([128, N], bf16)
            xt = singles.tile([128, NJ, KC, 512], bf16)
            for j in range(NJ):
                nc.sync.dma_start(out=xt[:, j, :, :], in_=xT[:, j, :, :])
            for j in range(NJ):
                js = slice(j * 512, (j + 1) * 512)
                nc.sync.dma_start(out=cost[:, js], in_=cosr[:, js])
                nc.sync.dma_start(out=sint[:, js], in_=sinr[:, js])

            # per-head persistent tiles (indexed by head)
            qr_t, kt_t, ksum_t, ksb_t, rsq_t, s24_t, r24_t, nsq_t = (
                {}, {}, {}, {}, {}, {}, {}, {})

            def chain_start(t):
                # qr: [e, n], partitions = 64 q-dims then 64 k-dims (raw
                # rotary outputs, unnormalized). kt: normalized k^.
                qr_t[t] = persist.tile([128, N], bf16, tag="qr", name=f"qr{t}")
                kt_t[t] = persist.tile([64, N], bf16, tag="kt", name=f"kt{t}")
                ksum_t[t] = persist.tile([64, NJ], f32, tag="ksum", name=f"ksum{t}")
                ksb_t[t] = persist.tile([64, 1], bf16, tag="ksb", name=f"ksb{t}")
                nsq_t[t] = persist.tile([2, N], f32, tag="nsq", name=f"nsq{t}")

            def chain_proj(t, j):
                # projection + rotary for chunk j: qr = qk*cos + rot(qk)*sin
                qr = qr_t[t]
                js = slice(j * 512, (j + 1) * 512)
                qk_ps = pchain.tile([128, 512], f32, tag="pq")
                rot_ps = pchain.tile([128, 512], f32, tag="pr")
                for k in range(KC):
                    nc.tensor.matmul(
                        qk_ps[:], lhsT=wqt[:, t, k, :], rhs=xt[:, j, k, :],
                        start=(k == 0), stop=(k == KC - 1),
                    )
                for k in range(KC):
                    nc.tensor.matmul(
                        rot_ps[:], lhsT=wrt[:, t, k, :], rhs=xt[:, j, k, :],
                        start=(k == 0), stop=(k == KC - 1),
                    )
                t1 = chain_pool.tile([128, 512], bf16, tag="t1")
                nc.vector.tensor_mul(t1[:], qk_ps[:], cost[:, js])
                t2 = chain_pool.tile([128, 512], bf16, tag="t2")
                nc.vector.tensor_mul(t2[:], rot_ps[:], sint[:, js])
                nc.vector.tensor_add(qr[:, js], t1[:], t2[:])
                # squared elements for the norm matmul (bf16 all-SBUF: 4x DVE)
                sq = chain_pool.tile([128, 512], bf16, tag="sq")
                nc.vector.tensor_mul(sq[:], qr[:, js], qr[:, js])
                return sq

            def chain_nsq(t, j, sq):
                # squared L2 norms along d (partition dim) via mask matmul;
                # DMA cannot read PSUM, so hop through SBUF on the (mostly
                # idle) scalar engine, then bounce to DRAM per chunk for the
                # partition-gather
                js = slice(j * 512, (j + 1) * 512)
                nsq_ps = pchain.tile([2, 512], f32, tag="pq")
                nc.tensor.matmul(
                    nsq_ps[:], lhsT=mask[:], rhs=sq[:], start=True, stop=True
                )
                nc.scalar.copy(out=nsq_t[t][:, js], in_=nsq_ps[:])
                nc.sync.dma_start(out=nsqd[t, :, js], in_=nsq_t[t][:, js])

            def chain_norm(t):
                # rsq cols 0:16 = (1/|q|)*2^24/64 per q-tile (partition-
                # gathered), cols 16:32 = 1/|k|; exact DVE reciprocal + ACT
                # sqrt
                nst = small.tile([128, 2 * NT], f32, tag="nst")
                nc.sync.dma_start(
                    out=nst[:, 0:NT], in_=nsqd[t, 0].rearrange("(i p) -> p i", p=128)
                )
                nc.sync.dma_start(
                    out=nst[:, NT:2 * NT],
                    in_=nsqd[t, 1].rearrange("(i p) -> p i", p=128),
                )
                rc = small.tile([128, 2 * NT], f32, tag="rc")
                nc.vector.reciprocal(rc[:], nst[:])
                rsq = persist.tile([128, 2 * NT], bf16, tag="rsq", name=f"rsq{t}")
                nc.scalar.activation(out=rsq[:], in_=rc[:], func=AF.Sqrt)
                rsq_t[t] = rsq
                # 1/|k| back to row layout for the knorm broadcast
                nc.sync.dma_start(
                    out=rkd[t].rearrange("(i p) -> p i", p=128),
                    in_=rsq[:, NT:2 * NT],
                )

            def chain_knorm(t, j):
                # kt = (k-half of qr, shifted to base partition 0) * 1/|k|;
                # accumulate Ksum chunk via accum_out
                qr, kt = qr_t[t], kt_t[t]
                js = slice(j * 512, (j + 1) * 512)
                ks = chain_pool.tile([64, 512], bf16, tag="ks")
                nc.sync.dma_start(out=ks[:], in_=qr[64:128, js])
                bc = chain_pool.tile([64, 512], bf16, tag="bc")
                nc.sync.dma_start(out=bc[:], in_=rkd[t:t + 1, js].to_broadcast([64, 512]))
                nc.vector.scalar_tensor_tensor(
                    out=kt[:, js], in0=ks[:], scalar=1.0, in1=bc[:],
                    op0=OP.mult, op1=OP.mult,
                    accum_out=ksum_t[t][:, j:j + 1],
                )

            def chain_finish(t):
                # Ksum (bf16 [64,1]) then per-q-tile scales:
                #   u_i   = q_i . Ksum                       (16 tiny matmuls)
                #   den_i = 2048 + u_i * rq24_i * 2^-24
                #   r_i   = 1/den_i
                #   s24   = r_i * rq24_i        (= s_i * 2^24)
                #   r24   = r_i * 2^24 - 8192   (= (r_i - 1/N) * 2^24)
                ksum, ksb = ksum_t[t], ksb_t[t]
                s1 = small.tile([64, 1], f32, tag="s1")
                nc.vector.tensor_add(s1[:], ksum[:, 0:1], ksum[:, 1:2])
                s2 = small.tile([64, 1], f32, tag="s2")
                nc.vector.tensor_add(s2[:], ksum[:, 2:3], ksum[:, 3:4])
                nc.vector.tensor_add(ksb[:], s1[:], s2[:])

                qr = qr_t[t]
                u_ps = pchain.tile([128, NT], f32, tag="pr")
                for i in range(NT):
                    nc.tensor.matmul(
                        u_ps[:, i:i + 1], lhsT=qr[0:64, i * 128:(i + 1) * 128],
                        rhs=ksb[:], start=True, stop=True,
                    )
                rq24 = rsq_t[t][:, 0:NT]
                tu = small.tile([128, NT], f32, tag="tu")
                nc.vector.tensor_mul(tu[:], u_ps[:], rq24)
                den = small.tile([128, NT], f32, tag="den")
                nc.vector.tensor_scalar(
                    out=den[:], in0=tu[:], scalar1=2.0 ** -24, scalar2=float(N),
                    op0=OP.mult, op1=OP.add,
                )
                rr = small.tile([128, NT], f32, tag="rr")
                nc.vector.reciprocal(rr[:], den[:])
                s24 = persist.tile([128, NT], f32, tag="s24", name=f"s24{t}")
                nc.vector.tensor_mul(s24[:], rr[:], rq24)
                r24 = persist.tile([128, NT], f32, tag="r24", name=f"r24{t}")
                nc.vector.tensor_scalar(
                    out=r24[:], in0=rr[:], scalar1=OSCALE, scalar2=-OSCALE / N,
                    op0=OP.mult, op1=OP.add,
                )
                s24_t[t], r24_t[t] = s24, r24

            ep_idx = [0]

            def scores_tile(t, i):
                # two [128,1024] PSUM halves; epilogue = fused scale+bias to
                # fp8 deltas, split ACT:DVE ~ 2:1
                qr, kt = qr_t[t], kt_t[t]
                s24, r24 = s24_t[t], r24_t[t]
                isl = slice(i * 128, (i + 1) * 128)
                ot = out_pool.tile([128, N], f8, tag="ot")
                for hlf in range(2):
                    hs = slice(hlf * 1024, (hlf + 1) * 1024)
                    sc_ps = psc.tile([128, 1024], f32, tag="sc")
                    for j2 in range(2):
                        js2 = slice(hlf * 1024 + j2 * 512, hlf * 1024 + (j2 + 1) * 512)
                        nc.tensor.matmul(
                            sc_ps[:, j2 * 512:(j2 + 1) * 512],
                            lhsT=qr[0:64, isl], rhs=kt[:, js2],
                            start=True, stop=True,
                        )
                    if ep_idx[0] % 3 != 2:
                        nc.scalar.activation(
                            out=ot[:, hs], in_=sc_ps[:], func=AF.Identity,
                            scale=s24[:, i:i + 1], bias=r24[:, i:i + 1],
                        )
                    else:
                        nc.vector.tensor_scalar(
                            out=ot[:, hs], in0=sc_ps[:],
                            scalar1=s24[:, i:i + 1], scalar2=r24[:, i:i + 1],
                            op0=OP.mult, op1=OP.add,
                        )
                    ep_idx[0] += 1
                nc.sync.dma_start(out=out[t, isl, :], in_=ot[:])

            # software pipeline: keep the PE queue dense. Head-0 projection,
            # then head-1 projection runs while head-0's norm pipeline
            # (gather/rsqrt/knorm, all DVE/ACT/DMA) completes; head-1's norm
            # interleaves into head-0's scores loop.
            chain_start(0)
            sqs = [chain_proj(0, j) for j in range(NJ)]
            for j in range(NJ):
                chain_nsq(0, j, sqs[j])
            chain_start(1)
            sqs1 = [chain_proj(1, j) for j in range(NJ)]
            chain_norm(0)
            for j in range(NJ):
                chain_knorm(0, j)
            chain_finish(0)
            for j in range(NJ):
                chain_nsq(1, j, sqs1[j])
            for i in range(NT):
                scores_tile(0, i)
                if i == 1:
                    chain_norm(1)
                elif i in (3, 4, 5, 6):
                    chain_knorm(1, i - 3)
                elif i == 8:
                    chain_finish(1)
            for i in range(NT):
                scores_tile(1, i)

    nc.compile()
    return nc


def _get_nc():
    if "nc" not in _CACHE:
        _CACHE["nc"] = _build_nc()
    return _CACHE["nc"]


def _prep_inputs(x, rotary_cos, rotary_sin, W_qk):
    bf16 = ml_dtypes.bfloat16
    x = np.asarray(x, dtype=np.float32)
    cos = np.asarray(rotary_cos, dtype=np.float32)
    sin = np.asarray(rotary_sin, dtype=np.float32)
    W = np.asarray(W_qk, dtype=np.float32)

    cosr = np.concatenate([cos.T, cos.T], axis=0).astype(bf16)  # [128, N]
    sinr = np.concatenate([sin.T, sin.T], axis=0).astype(bf16)
    maskt = np.zeros((128, 2), dtype=bf16)
    maskt[0:64, 0] = MASK_Q
    maskt[64:128, 1] = 1.0

    # per-head weight lhsT chunks (and rotate_half-permuted variant),
    # stored partition-major: [p, head, kc, m]
    wq_h = np.empty((H, KC, 128, 128), dtype=np.float32)
    wr_h = np.empty((H, KC, 128, 128), dtype=np.float32)
    for h in range(H):
        wcat = np.concatenate(
            [W[h * D:(h + 1) * D], W[C + h * D:C + (h + 1) * D]], axis=0
        )  # [128, 512]
        wrot = np.empty_like(wcat)
        wrot[0:32] = -wcat[32:64]
        wrot[32:64] = wcat[0:32]
        wrot[64:96] = -wcat[96:128]
        wrot[96:128] = wcat[64:96]
        wq_h[h] = wcat.T.reshape(KC, 128, 128)
        wr_h[h] = wrot.T.reshape(KC, 128, 128)

    # xT partition-major chunked: [p, j, kc, nn]
    xTb = []
    for b in range(B):
        xT = x[b].T  # [C, N]
        xTb.append(np.ascontiguousarray(
            xT.reshape(KC, 128, NJ, 512).transpose(1, 2, 0, 3)
        ).astype(bf16))

    in_maps = []
    for core in range(NCORES):
        b = core // 4
        h0 = (core % 4) * HPC
        wqc = np.ascontiguousarray(
            wq_h[h0:h0 + HPC].transpose(2, 0, 1, 3)
        ).astype(bf16)  # [128, HPC, KC, 128]
        wrc = np.ascontiguousarray(
            wr_h[h0:h0 + HPC].transpose(2, 0, 1, 3)
        ).astype(bf16)
        in_maps.append({
            "xT": xTb[b],
            "wq": wqc,
            "wr": wrc,
            "cosr": cosr,
            "sinr": sinr,
            "maskt": maskt,
        })
    return in_maps


def run(x, rotary_cos, rotary_sin, W_qk, trace=False):
    from concourse.bass_utils import run_bass_kernel_spmd

    nc = _get_nc()
    in_maps = _prep_inputs(x, rotary_cos, rotary_sin, W_qk)
    res = run_bass_kernel_spmd(nc, in_maps, list(range(NCORES)), trace=trace)
    full = np.empty((B, H, N, N), dtype=np.float32)
    inv_scale = np.float32(1.0 / OSCALE)
    mean = np.float32(1.0 / N)
    for core in range(NCORES):
        b = core // 4
        h0 = (core % 4) * HPC
        dl = np.asarray(res.results[core]["out"]).astype(np.float32)
        for t in range(HPC):
            full[b, h0 + t] = dl[t] * inv_scale + mean
    return full, res


def kernel(x, rotary_cos, rotary_sin, W_qk):
    full, _ = run(x, rotary_cos, rotary_sin, W_qk, trace=False)
    return full
